# revision 1
# baseline (speedup 1.0000x reference)
"""Trainium2 Bass kernel for the AttnBlock problem.

Contract: kernel(**inputs) takes the FULL unsharded inputs (numpy, keyed as in
setup_inputs) and returns the FULL output [32, 512, 512] (fp32).

Strategy: data-parallel over batch B=32 across 8 NeuronCores (4 samples/core,
weights replicated). Per sample everything is kept in [feature-on-partition,
t-on-free] layout:
  conv (weight-norm, K=3) -> GLU -> y[c,t]
  qT[d,t] = fc1T.T @ y + (word_embed + fc1_b)^T
  G[n,c]  = af.T @ fc2_w.T   (fc2 folded through the n=196 bottleneck:
            o = fc2 @ ctx^T = G^T @ attnT, saving 8 matmuls/sample)
  scores[t,n] = qT_tile.T @ af_pad   (n padded 196->256 so fp32r runs 1 cyc/row)
  softmax over free dim n, then PE-transpose of normalized attn -> attnT[n,t]
  o[c,t] = G.T @ attnT ; out = o + fc2_b + y + x
All matmuls use float32r (full PE rate for free dim >= 256). Host precomputes
the weight-norm conv weight and all transposed layouts (cheap, O(params)).
"""

import os
import sys

import numpy as np

for _p in ("/opt/trn_rl_repo",):
    if os.path.isdir(_p) and _p not in sys.path:
        sys.path.insert(0, _p)

from contextlib import ExitStack

import concourse.bass as bass
import concourse.tile as tile
from concourse import bacc, mybir
from concourse import bass_utils
from concourse.masks import make_identity

F32 = mybir.dt.float32
F32R = mybir.dt.float32r
F16 = mybir.dt.float16
AF = mybir.ActivationFunctionType
OP = mybir.AluOpType
AX = mybir.AxisListType

B, CIN, T = 32, 512, 512
COUT, KW = 1024, 3
WORD, D = 512, 512
HW, HWP = 196, 256
N_CORES = 8
BL = B // N_CORES  # samples per core

_CACHE = {}


def _emit_sample_A(nc, st, s, w):
    """Phase A for sample s: input DMAs, conv+GLU -> y, qT, G, scores+softmax."""
    p = st[s] = {}

    # ---- per-sample input DMAs (for s=0, defer non-conv inputs so the
    #      weight + x DMAs get full HBM bandwidth before the first matmul) ----
    xpad = w["xpool"].tile([128, 4, T + 2], F16, name="xpad", tag="xpad")
    nc.gpsimd.memset(xpad[:, :, 0:2], 0.0)
    if s == 0:
        # interleave per-ci x chunks with the pair-0 weight chunks so the
        # first conv matmuls start as soon as their operands land
        for ci in range(4):
            nc.sync.dma_start(
                out=xpad[:, ci, 2 : T + 2],
                in_=w["x_d"][s, ci * 128 : (ci + 1) * 128, :],
            )
            nc.sync.dma_start(
                out=w["wt"][ci][:, 0, :, :],
                in_=w["wt_d"][0, ci * 128 : (ci + 1) * 128, :, :],
            )
        w["load_weights"](skip_pair0=True)
    else:
        nc.sync.dma_start(
            out=xpad[:, :, 2 : T + 2],
            in_=w["x_d"][s].rearrange("(c p) t -> p c t", p=128),
        )
    p["xpad"] = xpad

    def load_rest():
        xr = w["xrpool"].tile([128, 4, T], F32, name="xr", tag="xr")
        nc.sync.dma_start(
            out=xr[:], in_=w["xr_d"][s].rearrange("(c p) t -> p c t", p=128)
        )
        p["xr"] = xr
        wet = w["wepool"].tile([128, 4, T], F32, name="wet", tag="wet")
        nc.sync.dma_start(
            out=wet[:], in_=w["wet_d"][s].rearrange("(c p) t -> p c t", p=128)
        )
        afp = w["afpool"].tile([128, 4, HWP], F32R, name="afp", tag="afp")
        nc.sync.dma_start(
            out=afp[:], in_=w["afp_d"][s].rearrange("(c p) n -> p c n", p=128)
        )
        return wet, afp

    if s != 0:
        wet, afp = load_rest()

    # ---- conv + GLU -> y ----
    y = w["ypool"].tile([128, 4, T], F32R, name="y", tag="y")
    p["y"] = y
    for i in range(4):  # GLU pair: co tile i (a-half) with co tile i+4 (b-half)
        if s == 0 and i == 1:
            # pair-0 matmuls are emitted; start the remaining input loads now
            # so they land under the pair-1..3 compute window
            w["post_conv_0"]()
            wet, afp = load_rest()
        ps_a = w["ps512"].tile([128, T], F32, name="mm", tag="mm")
        ps_b = w["ps512"].tile([128, T], F32, name="mm", tag="mm")
        for half, ps in ((0, ps_a), (1, ps_b)):
            for ci in range(4):
                for k in range(KW):
                    nc.tensor.matmul(
                        ps[:],
                        w["wt"][ci][:, i, k, half * 128 : (half + 1) * 128],
                        xpad[:, ci, k : k + T],
                        start=ci == 0 and k == 0,
                        stop=ci == 3 and k == KW - 1,
                    )
        sig = w["sigpool"].tile([128, T], F32, name="sig", tag="sig")
        nc.scalar.activation(
            sig[:], ps_b[:], AF.Sigmoid, bias=w["cb"][:, i + 4 : i + 5], scale=1.0
        )
        # y_i = (conv_a + bias_a) * sigmoid(conv_b + bias_b)
        nc.vector.scalar_tensor_tensor(
            out=y[:, i, :], in0=ps_a[:], scalar=w["cb"][:, i : i + 1], in1=sig[:],
            op0=OP.add, op1=OP.mult,
        )

    # yx = y + x precomputed here so output assembly is a single DVE op
    yx = w["yxpool"].tile([128, 4, T], F32, name="yx", tag="yx")
    for i in range(4):
        nc.vector.tensor_add(yx[:, i, :], y[:, i, :], p["xr"][:, i, :])
    p["yx"] = yx

    # ---- qT[d,t] = fc1T.T @ y + weT ----
    qt = w["qpool"].tile([128, 4, T], F32R, name="qt", tag="qt")
    for dt_ in range(4):
        ps = w["ps512"].tile([128, T], F32, name="mm", tag="mm")
        for cc in range(4):
            nc.tensor.matmul(
                ps[:],
                w["fc1t"][:, cc, dt_ * 128 : (dt_ + 1) * 128],
                y[:, cc, :],
                start=cc == 0,
                stop=cc == 3,
            )
        nc.vector.tensor_add(qt[:, dt_, :], ps[:], wet[:, dt_, :])

    # ---- scores + softmax (normalized attn weights in [t, n]) ----
    attn_tiles = []
    for tt in range(4):
        ps_s = w["ps256"].tile([128, HWP], F32, name="sc", tag="sc")
        for dd in range(4):
            nc.tensor.matmul(
                ps_s[:],
                qt[:, dd, tt * 128 : (tt + 1) * 128],
                afp[:, dd, :],
                start=dd == 0,
                stop=dd == 3,
            )
        nmax = w["colpool"].tile([128, 1], F32, name="col", tag="col")
        nc.vector.reduce_max(out=nmax[:], in_=ps_s[:, 0:HW], axis=AX.X, negate=True)
        attn_t = w["attnpool"].tile([128, HW], F32, name="attn", tag="attn")
        rsum = w["colpool"].tile([128, 1], F32, name="col", tag="col")
        nc.scalar.activation(
            attn_t[:], ps_s[:, 0:HW], AF.Exp, bias=nmax[:], scale=1.0, accum_out=rsum[:]
        )
        rinv = w["colpool"].tile([128, 1], F32, name="col", tag="col")
        nc.vector.reciprocal(rinv[:], rsum[:])
        nc.vector.tensor_scalar_mul(attn_t[:], attn_t[:], rinv[:])
        attn_tiles.append(attn_t)
    p["attn"] = attn_tiles

    # ---- G[n,c] = sum_d af[d,n] * fc2T[d,c] -- emitted AFTER scores so these
    #      softmax-independent matmuls cover the softmax latency on PE ----
    g_sb = w["gpool"].tile([128, 2, WORD], F32R, name="g", tag="g")
    nc.gpsimd.memset(g_sb[64:128, 1, :].bitcast(F32), 0.0)
    for nch in range(2):
        nsz = 128 if nch == 0 else HW - 128
        g_ps = w["psT"].tile([128, WORD], F32, name="tp", tag="tp")
        for dd in range(4):
            nc.tensor.matmul(
                g_ps[0:nsz, :],
                afp[:, dd, nch * 128 : nch * 128 + nsz],
                w["fc2t"][:, dd, :],
                start=dd == 0,
                stop=dd == 3,
            )
        nc.vector.tensor_copy(g_sb[0:nsz, nch, :], g_ps[0:nsz, :])
    p["g"] = g_sb


def _emit_sample_TC(nc, st, s, w):
    """Transpose attn[t,n] -> attnT[n,t] via PE, copy to SBUF."""
    p = st[s]
    attn_tiles = p["attn"]
    tps = [w["psT"].tile([128, T], F32, name="tp", tag="tp") for _ in range(2)]
    for tt in range(4):
        for nch in range(2):
            nsz = 128 if nch == 0 else HW - 128
            nc.tensor.transpose(
                tps[nch][0:nsz, tt * 128 : (tt + 1) * 128],
                attn_tiles[tt][:, nch * 128 : nch * 128 + nsz],
                w["ident"][:],
            )
    at = w["atpool"].tile([128, 2, T], F32R, name="at", tag="at")
    nc.gpsimd.memset(at[64:128, 1, :].bitcast(F32), 0.0)
    nc.vector.tensor_copy(at[:, 0, :], tps[0][:])
    nc.vector.tensor_copy(at[0 : HW - 128, 1, :], tps[1][0 : HW - 128, :])
    p["at"] = at


def _emit_sample_O(nc, st, s, w):
    """o[c,t] = G.T @ attnT ; out = o + fc2_b + (y + x) ; store."""
    p = st[s]
    g_sb, at, yx = p["g"], p["at"], p["yx"]
    for ct in range(4):
        ps = w["ps512"].tile([128, T], F32, name="mm", tag="mm")
        for nch in range(2):
            nc.tensor.matmul(
                ps[:],
                g_sb[:, nch, ct * 128 : (ct + 1) * 128],
                at[:, nch, :],
                start=nch == 0,
                stop=nch == 1,
            )
        tmp = w["opool"].tile([128, T], F32, name="tmp", tag="tmp")
        nc.vector.scalar_tensor_tensor(
            out=tmp[:], in0=ps[:], scalar=w["f2b"][:, ct : ct + 1], in1=yx[:, ct, :],
            op0=OP.add, op1=OP.add,
        )
        nc.sync.dma_start(out=w["out_d"][s, ct * 128 : (ct + 1) * 128, :], in_=tmp[:])


def build_nc():
    """Build and compile the per-core Bass program (shared by all 8 cores)."""
    nc = bacc.Bacc("TRN2", target_bir_lowering=False, debug=False, num_devices=N_CORES)
    w = {}
    w["x_d"] = nc.dram_tensor("x", [BL, CIN, T], F16, kind="ExternalInput").ap()
    w["xr_d"] = nc.dram_tensor("xr", [BL, CIN, T], F32, kind="ExternalInput").ap()
    w["wet_d"] = nc.dram_tensor("wet", [BL, D, T], F32, kind="ExternalInput").ap()
    w["afp_d"] = nc.dram_tensor("afp", [BL, D, HWP], F32R, kind="ExternalInput").ap()
    w["wt_d"] = nc.dram_tensor("wt", [4, CIN, KW, 256], F16, kind="ExternalInput").ap()
    w["fc1t_d"] = nc.dram_tensor("fc1t", [WORD, D], F32R, kind="ExternalInput").ap()
    w["fc2t_d"] = nc.dram_tensor("fc2t", [D, WORD], F32R, kind="ExternalInput").ap()
    w["cb_d"] = nc.dram_tensor("cb", [128, 8], F32, kind="ExternalInput").ap()
    w["f2b_d"] = nc.dram_tensor("f2b", [128, 4], F32, kind="ExternalInput").ap()
    w["out_d"] = nc.dram_tensor("out", [BL, WORD, T], F32, kind="ExternalOutput").ap()

    with tile.TileContext(nc) as tc, ExitStack() as ctx:
        pool = lambda name, bufs, **kw: ctx.enter_context(
            tc.tile_pool(name=name, bufs=bufs, **kw)
        )
        wpool = pool("wts", 1)
        cpool = pool("consts", 1)
        w["xpool"] = pool("xp", 3)
        w["xrpool"] = pool("xrp", 2)
        w["yxpool"] = pool("yxp", 2)
        w["wepool"] = pool("wep", 2)
        w["afpool"] = pool("afp", 2)
        w["ypool"] = pool("yp", 2)
        w["qpool"] = pool("qp", 1)
        w["gpool"] = pool("gp", 2)
        w["attnpool"] = pool("attnp", 8)
        w["sigpool"] = pool("sigp", 2)
        w["atpool"] = pool("atp", 2)
        w["opool"] = pool("op", 4)
        w["colpool"] = pool("colp", 16)
        w["ps512"] = pool("ps512", 4, space="PSUM")
        w["ps256"] = pool("ps256", 2, space="PSUM")
        w["psT"] = pool("psT", 2, space="PSUM")

        # resident weights. wt is stored pair-major ([pair, ci, k, 256]) and
        # loaded pair-by-pair so conv pair 0 starts after ~2MB of DMA instead
        # of the full weight set; fc1t/fc2t are deferred past sample-0 conv.
        w["wt"] = [
            wpool.tile([128, 4, KW, 256], F16, name=f"wt{c}", tag=f"wt{c}")
            for c in range(4)
        ]
        w["fc1t"] = wpool.tile([128, 4, D], F32R, name="fc1t", tag="fc1t")
        w["fc2t"] = wpool.tile([128, 4, WORD], F32R, name="fc2t", tag="fc2t")
        w["cb"] = cpool.tile([128, 8], F32, name="cb", tag="cb")
        w["f2b"] = cpool.tile([128, 4], F32, name="f2b", tag="f2b")
        w["ident"] = cpool.tile([128, 128], F32, name="ident", tag="ident")

        def load_weights(skip_pair0=False):
            for i in range(1 if skip_pair0 else 0, 4):
                for c in range(4):
                    nc.sync.dma_start(
                        out=w["wt"][c][:, i, :, :],
                        in_=w["wt_d"][i, c * 128 : (c + 1) * 128, :, :],
                    )
            nc.sync.dma_start(out=w["cb"][:], in_=w["cb_d"][:])
            nc.sync.dma_start(out=w["f2b"][:], in_=w["f2b_d"][:])
            make_identity(nc, w["ident"][:])

        w["load_weights"] = load_weights

        def post_conv_0():
            nc.sync.dma_start(
                out=w["fc1t"][:], in_=w["fc1t_d"].rearrange("(c p) d -> p c d", p=128)
            )
            nc.sync.dma_start(
                out=w["fc2t"][:], in_=w["fc2t_d"].rearrange("(c p) d -> p c d", p=128)
            )

        w["post_conv_0"] = post_conv_0

        # software-pipelined emission: per steady-state sample the PE stream is
        # [conv qT scores G] [o-matmuls of s-1] [transposes of s], so the
        # softmax of sample s is hidden under G_s + O_{s-1}.
        st = {}
        _emit_sample_A(nc, st, 0, w)
        _emit_sample_TC(nc, st, 0, w)
        for s in range(1, BL):
            _emit_sample_A(nc, st, s, w)
            _emit_sample_O(nc, st, s - 1, w)
            _emit_sample_TC(nc, st, s, w)
        _emit_sample_O(nc, st, BL - 1, w)

    nc.compile()
    return nc


def prep_inputs(x, word_embed, img_conv, conv_v, conv_g, conv_b, fc1_w, fc1_b, fc2_w, fc2_b):
    """Host-side weight-norm + layout prep. Returns per-core input maps."""
    x = np.asarray(x, dtype=np.float32)
    word_embed = np.asarray(word_embed, dtype=np.float32)
    img_conv = np.asarray(img_conv, dtype=np.float32)
    conv_v = np.asarray(conv_v, dtype=np.float32)
    conv_g = np.asarray(conv_g, dtype=np.float32)
    conv_b = np.asarray(conv_b, dtype=np.float32)
    fc1_w = np.asarray(fc1_w, dtype=np.float32)
    fc1_b = np.asarray(fc1_b, dtype=np.float32)
    fc2_w = np.asarray(fc2_w, dtype=np.float32)
    fc2_b = np.asarray(fc2_b, dtype=np.float32)

    v_norm = np.sqrt(np.sum(conv_v * conv_v, axis=(1, 2), keepdims=True))
    wconv = conv_g[:, None, None] * conv_v / v_norm  # [COUT, CIN, KW]
    wtf = wconv.transpose(1, 2, 0).astype(np.float16)  # [CIN, KW, COUT]
    wt = np.ascontiguousarray(
        np.stack(
            [
                np.concatenate(
                    [wtf[:, :, i * 128 : (i + 1) * 128],
                     wtf[:, :, (i + 4) * 128 : (i + 5) * 128]],
                    axis=-1,
                )
                for i in range(4)
            ]
        )
    )  # [4, CIN, KW, 256] pair-major
    fc1t = np.ascontiguousarray(fc1_w.T)  # [c, d]
    fc2t = np.ascontiguousarray(fc2_w.T)  # [d, c]
    cb = np.ascontiguousarray(conv_b.reshape(8, 128).T)  # [128, 8]
    f2b = np.ascontiguousarray(fc2_b.reshape(4, 128).T)  # [128, 4]

    wet = np.ascontiguousarray(
        (word_embed + fc1_b[None, None, :]).transpose(0, 2, 1)
    )  # [B, d, t]
    af = img_conv.reshape(B, D, HW)
    afp = np.zeros((B, D, HWP), dtype=np.float32)
    afp[:, :, :HW] = af

    in_maps = []
    for c in range(N_CORES):
        sl = slice(c * BL, (c + 1) * BL)
        in_maps.append(
            {
                "x": np.ascontiguousarray(x[sl].astype(np.float16)),
                "xr": np.ascontiguousarray(x[sl]),
                "wet": np.ascontiguousarray(wet[sl]),
                "afp": np.ascontiguousarray(afp[sl]),
                "wt": wt,
                "fc1t": fc1t,
                "fc2t": fc2t,
                "cb": cb,
                "f2b": f2b,
            }
        )
    return in_maps


def _install_ntff_shim():
    """Make run_bass_kernel_spmd(trace=True) work under axon in this image."""
    import types

    if "antenv.axon_hooks" in sys.modules:
        return True
    try:
        m = types.ModuleType("antenv.axon_hooks")
        _hooks = {}

        def set_axon_ntff_profile_hook(h):
            _hooks["h"] = h

        def get_axon_ntff_profile_hook():
            return _hooks.get("h")

        m.set_axon_ntff_profile_hook = set_axon_ntff_profile_hook
        m.get_axon_ntff_profile_hook = get_axon_ntff_profile_hook
        sys.modules["antenv.axon_hooks"] = m
        import antenv

        antenv.axon_hooks = m
        from trn_agent_boot.trn_boot import _ntff_profile_via_ctypes

        hook = _ntff_profile_via_ctypes("/opt/axon/libaxon_pjrt.so")
        set_axon_ntff_profile_hook(hook)
        return hook is not None
    except Exception:
        return False


def kernel(x, word_embed, img_conv, prev_attn=None, conv_v=None, conv_g=None,
           conv_b=None, fc1_w=None, fc1_b=None, fc2_w=None, fc2_b=None):
    if "nc" not in _CACHE:
        _CACHE["nc"] = build_nc()
    nc = _CACHE["nc"]

    in_maps = prep_inputs(
        x, word_embed, img_conv, conv_v, conv_g, conv_b, fc1_w, fc1_b, fc2_w, fc2_b
    )

    trace = bool(os.environ.get("ATTN_BASS_TRACE"))
    if trace:
        trace = _install_ntff_shim()
    res = bass_utils.run_bass_kernel_spmd(
        nc, in_maps, core_ids=list(range(N_CORES)), trace=trace
    )
    if trace:
        _CACHE["exec_time_ns"] = res.exec_time_ns
        _CACHE["last_results"] = res

    out = np.concatenate([res.results[i]["out"] for i in range(N_CORES)], axis=0)
    return out.astype(np.float32)



# revision 6
# speedup vs baseline: 1.0455x; 1.0455x over previous
"""Trainium2 Bass kernel for the AttnBlock problem.

Contract: kernel(**inputs) takes the FULL unsharded inputs (numpy, keyed as in
setup_inputs) and returns the FULL output [32, 512, 512] (fp32).

Strategy: data-parallel over batch B=32 across 8 NeuronCores (4 samples/core,
weights replicated). Per sample everything is kept in [feature-on-partition,
t-on-free] layout:
  conv (weight-norm, K=3) -> GLU -> y[c,t]
  qT[d,t] = fc1T.T @ y + (word_embed + fc1_b)^T
  G[n,c]  = af.T @ fc2_w.T   (fc2 folded through the n=196 bottleneck:
            o = fc2 @ ctx^T = G^T @ attnT, saving 8 matmuls/sample)
  scores[t,n] = qT_tile.T @ af_pad   (n padded 196->256 so fp32r runs 1 cyc/row)
  softmax over free dim n, then PE-transpose of normalized attn -> attnT[n,t]
  o[c,t] = G.T @ attnT ; out = o + fc2_b + y + x
All matmuls use float32r (full PE rate for free dim >= 256) or f16 (conv).
Host precomputes the weight-norm conv weight and p-major DMA layouts.

v2 over baseline:
  - ~5us of dummy warm-up matmuls at t=0 so the PE HAM clock-gate reaches
    2.4 GHz before the first real conv matmul (cold window is ~3.4us at
    1.2 GHz and otherwise lands on sample-0's conv).
  - p-major host layouts ([128, 4, T] per sample) -> 4KB contiguous
    per-partition DMA lines instead of 1KB, and the head-critical loads
    (x s0 + conv weights pair0) are split across both HW DMA queues
    (sync + scalar engines).
  - residual add uses the f16 x already loaded for the conv (the separate
    f32 copy of x is gone: -4MB DMA per core).
  - dense tail: for the last sample the O-matmuls of s-2, G, and the attn
    transposes are interleaved between the score tiles so the PE never
    idles long enough to re-trigger the HAM throttle during the epilogue.
  - fewer tile-pool buffers (teardown emits per-semaphore bookkeeping).
"""

import os
import sys

import numpy as np

for _p in ("/opt/trn_rl_repo",):
    if os.path.isdir(_p) and _p not in sys.path:
        sys.path.insert(0, _p)

from contextlib import ExitStack

import concourse.bass as bass
import concourse.tile as tile
from concourse import bacc, mybir
from concourse import bass_utils
from concourse.masks import make_identity

F32 = mybir.dt.float32
F32R = mybir.dt.float32r
F16 = mybir.dt.float16
AF = mybir.ActivationFunctionType
OP = mybir.AluOpType
AX = mybir.AxisListType

B, CIN, T = 32, 512, 512
COUT, KW = 1024, 3
WORD, D = 512, 512
HW, HWP = 196, 256
N_CORES = 8
BL = B // N_CORES  # samples per core

_CACHE = {}


def _emit_conv(nc, st, s, w):
    """Input DMAs + conv + GLU -> y, yx for sample s."""
    p = st[s] = {}

    xpad = w["xpool"].tile([128, 4, T + 2], F16, name="xpad", tag="xpad")
    nc.gpsimd.memset(xpad[:, :, 0:2], 0.0)
    if s == 0:
        # head-critical: split x(s0) and wt pair-0 across both DMA queues
        nc.sync.dma_start(out=xpad[:, 0:2, 2 : T + 2], in_=w["x_d"][s, :, 0:2, :])
        nc.scalar.dma_start(out=xpad[:, 2:4, 2 : T + 2], in_=w["x_d"][s, :, 2:4, :])
        for ci in range(4):
            eng = nc.sync if ci < 2 else nc.scalar
            eng.dma_start(
                out=w["wt"][ci][:, 0, :, :],
                in_=w["wt_d"][0, ci * 128 : (ci + 1) * 128, :, :],
            )
    else:
        nc.sync.dma_start(out=xpad[:, :, 2 : T + 2], in_=w["x_d"][s])
    p["xpad"] = xpad

    def load_rest():
        wet = w["wepool"].tile([128, 4, T], F32, name="wet", tag="wet")
        nc.scalar.dma_start(out=wet[:], in_=w["wet_d"][s])
        afp = w["afpool"].tile([128, 4, HWP], F32R, name="afp", tag="afp")
        nc.scalar.dma_start(out=afp[:], in_=w["afp_d"][s])
        return wet, afp

    if s == 0:
        # issue the fc-weight + s0 attention loads on the scalar queue NOW,
        # before the first sigmoid occupies that engine's queue slot; they
        # share HBM with the head-critical x/wt loads but drain on the
        # second DMA queue.
        w["post_conv_0"]()
    wet, afp = load_rest()

    y = w["ypool"].tile([128, 4, T], F32R, name="y", tag="y")
    p["y"] = y
    for i in range(4):  # GLU pair: co tile i (a-half) with co tile i+4 (b-half)
        if s == 0 and i == 1:
            # pair-0 matmuls are emitted; start the remaining weight loads so
            # they land under the pair-1..3 compute window
            w["load_weights"](skip_pair0=True)
        ps_a = w["ps512"].tile([128, T], F32, name="mm", tag="mm")
        ps_b = w["ps512"].tile([128, T], F32, name="mm", tag="mm")
        for half, ps in ((0, ps_a), (1, ps_b)):
            for ci in range(4):
                for k in range(KW):
                    nc.tensor.matmul(
                        ps[:],
                        w["wt"][ci][:, i, k, half * 128 : (half + 1) * 128],
                        xpad[:, ci, k : k + T],
                        start=ci == 0 and k == 0,
                        stop=ci == 3 and k == KW - 1,
                    )
        sig = w["sigpool"].tile([128, T], F32, name="sig", tag="sig")
        nc.scalar.activation(
            sig[:], ps_b[:], AF.Sigmoid, bias=w["cb"][:, i + 4 : i + 5], scale=1.0
        )
        # y_i = (conv_a + bias_a) * sigmoid(conv_b + bias_b)
        nc.vector.scalar_tensor_tensor(
            out=y[:, i, :], in0=ps_a[:], scalar=w["cb"][:, i : i + 1], in1=sig[:],
            op0=OP.add, op1=OP.mult,
        )

    # yx = y + x (f16 x reused from the conv input; rounding err ~2^-11 |x|)
    yx = w["yxpool"].tile([128, 4, T], F32, name="yx", tag="yx")
    for i in range(4):
        nc.vector.tensor_add(yx[:, i, :], y[:, i, :], xpad[:, i, 2 : T + 2])
    p["yx"] = yx
    p["wet"] = wet
    p["afp"] = afp


def _emit_fc1(nc, st, s, w):
    """qT[d,t] = fc1T.T @ y + weT."""
    p = st[s]
    y, wet = p["y"], p["wet"]
    qt = w["qpool"].tile([128, 4, T], F32R, name="qt", tag="qt")
    for dt_ in range(4):
        ps = w["ps512"].tile([128, T], F32, name="mm", tag="mm")
        for cc in range(4):
            nc.tensor.matmul(
                ps[:],
                w["fc1t"][:, cc, dt_ * 128 : (dt_ + 1) * 128],
                y[:, cc, :],
                start=cc == 0,
                stop=cc == 3,
            )
        nc.vector.tensor_add(qt[:, dt_, :], ps[:], wet[:, dt_, :])
    p["qt"] = qt


def _emit_scores_tile(nc, st, s, tt, w):
    """One t-tile of scores + softmax -> normalized attn tile."""
    p = st[s]
    qt, afp = p["qt"], p["afp"]
    ps_s = w["ps256"].tile([128, HWP], F32, name="sc", tag="sc")
    for dd in range(4):
        nc.tensor.matmul(
            ps_s[:],
            qt[:, dd, tt * 128 : (tt + 1) * 128],
            afp[:, dd, :],
            start=dd == 0,
            stop=dd == 3,
        )
    nmax = w["colpool"].tile([128, 1], F32, name="col", tag="col")
    nc.vector.reduce_max(out=nmax[:], in_=ps_s[:, 0:HW], axis=AX.X, negate=True)
    attn_t = w["attnpool"].tile([128, HW], F32, name="attn", tag="attn")
    rsum = w["colpool"].tile([128, 1], F32, name="col", tag="col")
    nc.scalar.activation(
        attn_t[:], ps_s[:, 0:HW], AF.Exp, bias=nmax[:], scale=1.0, accum_out=rsum[:]
    )
    rinv = w["colpool"].tile([128, 1], F32, name="col", tag="col")
    nc.vector.reciprocal(rinv[:], rsum[:])
    nc.vector.tensor_scalar_mul(attn_t[:], attn_t[:], rinv[:])
    p.setdefault("attn", []).append(attn_t)


def _emit_G(nc, st, s, w):
    """G[n,c] = sum_d af[d,n] * fc2T[d,c] (softmax-independent PE filler)."""
    p = st[s]
    afp = p["afp"]
    g_sb = w["gpool"].tile([128, 2, WORD], F32R, name="g", tag="g")
    nc.gpsimd.memset(g_sb[64:128, 1, :].bitcast(F32), 0.0)
    for nch in range(2):
        nsz = 128 if nch == 0 else HW - 128
        g_ps = w["psT"].tile([128, WORD], F32, name="tp", tag="tp")
        for dd in range(4):
            nc.tensor.matmul(
                g_ps[0:nsz, :],
                afp[:, dd, nch * 128 : nch * 128 + nsz],
                w["fc2t"][:, dd, :],
                start=dd == 0,
                stop=dd == 3,
            )
        nc.vector.tensor_copy(g_sb[0:nsz, nch, :], g_ps[0:nsz, :])
    p["g"] = g_sb


def _emit_sample_TC(nc, st, s, w):
    """Transpose attn[t,n] -> attnT[n,t] via PE, copy to SBUF."""
    p = st[s]
    attn_tiles = p["attn"]
    tps = [w["psT"].tile([128, T], F32, name="tp", tag="tp") for _ in range(2)]
    for tt in range(4):
        for nch in range(2):
            nsz = 128 if nch == 0 else HW - 128
            nc.tensor.transpose(
                tps[nch][0:nsz, tt * 128 : (tt + 1) * 128],
                attn_tiles[tt][:, nch * 128 : nch * 128 + nsz],
                w["ident"][:],
            )
    at = w["atpool"].tile([128, 2, T], F32R, name="at", tag="at")
    nc.gpsimd.memset(at[64:128, 1, :].bitcast(F32), 0.0)
    nc.vector.tensor_copy(at[:, 0, :], tps[0][:])
    nc.vector.tensor_copy(at[0 : HW - 128, 1, :], tps[1][0 : HW - 128, :])
    p["at"] = at


def _emit_sample_O(nc, st, s, w, cts=range(4)):
    """o[c,t] = G.T @ attnT ; out = o + fc2_b + (y + x) ; store."""
    p = st[s]
    g_sb, at, yx = p["g"], p["at"], p["yx"]
    for ct in cts:
        ps = w["ps512"].tile([128, T], F32, name="mm", tag="mm")
        for nch in range(2):
            nc.tensor.matmul(
                ps[:],
                g_sb[:, nch, ct * 128 : (ct + 1) * 128],
                at[:, nch, :],
                start=nch == 0,
                stop=nch == 1,
            )
        tmp = w["opool"].tile([128, T], F32, name="tmp", tag="tmp")
        nc.vector.scalar_tensor_tensor(
            out=tmp[:], in0=ps[:], scalar=w["f2b"][:, ct : ct + 1], in1=yx[:, ct, :],
            op0=OP.add, op1=OP.add,
        )
        nc.sync.dma_start(out=w["out_d"][s, ct * 128 : (ct + 1) * 128, :], in_=tmp[:])


def build_nc():
    """Build and compile the per-core Bass program (shared by all 8 cores)."""
    nc = bacc.Bacc("TRN2", target_bir_lowering=False, debug=False, num_devices=N_CORES)
    w = {}
    w["x_d"] = nc.dram_tensor("x", [BL, 128, 4, T], F16, kind="ExternalInput").ap()
    w["wet_d"] = nc.dram_tensor("wet", [BL, 128, 4, T], F32, kind="ExternalInput").ap()
    w["afp_d"] = nc.dram_tensor(
        "afp", [BL, 128, 4, HWP], F32R, kind="ExternalInput"
    ).ap()
    w["wt_d"] = nc.dram_tensor("wt", [4, CIN, KW, 256], F16, kind="ExternalInput").ap()
    w["fc1t_d"] = nc.dram_tensor("fc1t", [WORD, D], F32R, kind="ExternalInput").ap()
    w["fc2t_d"] = nc.dram_tensor("fc2t", [D, WORD], F32R, kind="ExternalInput").ap()
    w["cb_d"] = nc.dram_tensor("cb", [128, 8], F32, kind="ExternalInput").ap()
    w["f2b_d"] = nc.dram_tensor("f2b", [128, 4], F32, kind="ExternalInput").ap()
    w["out_d"] = nc.dram_tensor("out", [BL, WORD, T], F32, kind="ExternalOutput").ap()

    with tile.TileContext(nc) as tc, ExitStack() as ctx:
        pool = lambda name, bufs, **kw: ctx.enter_context(
            tc.tile_pool(name=name, bufs=bufs, **kw)
        )
        wpool = pool("wts", 1)
        cpool = pool("consts", 1)
        w["xpool"] = pool("xp", 2)
        w["yxpool"] = pool("yxp", 2)
        w["wepool"] = pool("wep", 2)
        w["afpool"] = pool("afp", 2)
        w["ypool"] = pool("yp", 2)
        w["qpool"] = pool("qp", 1)
        w["gpool"] = pool("gp", 2)
        w["attnpool"] = pool("attnp", 6)
        w["sigpool"] = pool("sigp", 2)
        w["atpool"] = pool("atp", 2)
        w["opool"] = pool("op", 3)
        w["colpool"] = pool("colp", 8)
        w["ps512"] = pool("ps512", 4, space="PSUM")
        w["ps256"] = pool("ps256", 2, space="PSUM")
        w["psT"] = pool("psT", 2, space="PSUM")

        w["wt"] = [
            wpool.tile([128, 4, KW, 256], F16, name=f"wt{c}", tag=f"wt{c}")
            for c in range(4)
        ]
        w["fc1t"] = wpool.tile([128, 4, D], F32R, name="fc1t", tag="fc1t")
        w["fc2t"] = wpool.tile([128, 4, WORD], F32R, name="fc2t", tag="fc2t")
        w["cb"] = cpool.tile([128, 8], F32, name="cb", tag="cb")
        w["f2b"] = cpool.tile([128, 4], F32, name="f2b", tag="f2b")
        w["ident"] = cpool.tile([128, 128], F32, name="ident", tag="ident")

        # ---- PE warm-up: ~5us of junk matmuls with no DMA dependency so the
        # HAM clock-gate releases (1.2 -> 2.4 GHz) while the head DMAs land.
        # The psum tile comes from the ps512 pool; it has no readers so its
        # buffer recycles as soon as conv needs it.
        mz = cpool.tile([128, T], F32R, name="mz", tag="mz")
        nc.gpsimd.memset(mz[:].bitcast(F32), 0.0)
        ps_warm = w["ps512"].tile([128, T], F32, name="mm", tag="mm")
        for _ in range(12):
            nc.tensor.matmul(ps_warm[:], mz[:, 0:128], mz[:], start=True, stop=True)

        nc.scalar.dma_start(out=w["cb"][:], in_=w["cb_d"][:])
        nc.scalar.dma_start(out=w["f2b"][:], in_=w["f2b_d"][:])
        make_identity(nc, w["ident"][:])

        def load_weights(skip_pair0=False):
            for i in range(1 if skip_pair0 else 0, 4):
                for c in range(4):
                    nc.sync.dma_start(
                        out=w["wt"][c][:, i, :, :],
                        in_=w["wt_d"][i, c * 128 : (c + 1) * 128, :, :],
                    )

        w["load_weights"] = load_weights

        def post_conv_0():
            nc.scalar.dma_start(
                out=w["fc1t"][:], in_=w["fc1t_d"].rearrange("(c p) d -> p c d", p=128)
            )
            nc.scalar.dma_start(
                out=w["fc2t"][:], in_=w["fc2t_d"].rearrange("(c p) d -> p c d", p=128)
            )

        w["post_conv_0"] = post_conv_0

        # software-pipelined emission: per steady-state sample the PE stream is
        # [conv qT scores G] [o-matmuls of s-1] [transposes of s], so the
        # softmax of sample s is hidden under G_s + O_{s-1}.
        st = {}
        for s in range(BL):
            _emit_conv(nc, st, s, w)
            _emit_fc1(nc, st, s, w)
            if s < BL - 1:
                for tt in range(4):
                    _emit_scores_tile(nc, st, s, tt, w)
                _emit_G(nc, st, s, w)
                if s > 0:
                    _emit_sample_O(nc, st, s - 1, w)
                _emit_sample_TC(nc, st, s, w)
            else:
                # dense tail: interleave the previous sample's O-matmuls and
                # this sample's G between the score tiles so the PE stays busy
                # under the softmax latency (prevents the HAM re-throttle and
                # the ~1us per-transpose stalls at the end of the program).
                _emit_scores_tile(nc, st, s, 0, w)
                _emit_scores_tile(nc, st, s, 1, w)
                _emit_sample_O(nc, st, s - 1, w, cts=(0, 1))
                _emit_scores_tile(nc, st, s, 2, w)
                _emit_sample_O(nc, st, s - 1, w, cts=(2,))
                _emit_scores_tile(nc, st, s, 3, w)
                _emit_sample_O(nc, st, s - 1, w, cts=(3,))
                _emit_G(nc, st, s, w)
                _emit_sample_TC(nc, st, s, w)
        _emit_sample_O(nc, st, BL - 1, w)

    nc.compile()
    return nc


def prep_inputs(x, word_embed, img_conv, conv_v, conv_g, conv_b, fc1_w, fc1_b, fc2_w, fc2_b):
    """Host-side weight-norm + p-major layout prep. Returns per-core input maps."""
    x = np.asarray(x, dtype=np.float32)
    word_embed = np.asarray(word_embed, dtype=np.float32)
    img_conv = np.asarray(img_conv, dtype=np.float32)
    conv_v = np.asarray(conv_v, dtype=np.float32)
    conv_g = np.asarray(conv_g, dtype=np.float32)
    conv_b = np.asarray(conv_b, dtype=np.float32)
    fc1_w = np.asarray(fc1_w, dtype=np.float32)
    fc1_b = np.asarray(fc1_b, dtype=np.float32)
    fc2_w = np.asarray(fc2_w, dtype=np.float32)
    fc2_b = np.asarray(fc2_b, dtype=np.float32)

    v_norm = np.sqrt(np.sum(conv_v * conv_v, axis=(1, 2), keepdims=True))
    wconv = conv_g[:, None, None] * conv_v / v_norm  # [COUT, CIN, KW]
    wtf = wconv.transpose(1, 2, 0).astype(np.float16)  # [CIN, KW, COUT]
    wt = np.ascontiguousarray(
        np.stack(
            [
                np.concatenate(
                    [wtf[:, :, i * 128 : (i + 1) * 128],
                     wtf[:, :, (i + 4) * 128 : (i + 5) * 128]],
                    axis=-1,
                )
                for i in range(4)
            ]
        )
    )  # [4, CIN, KW, 256] pair-major
    fc1t = np.ascontiguousarray(fc1_w.T)  # [c, d]
    fc2t = np.ascontiguousarray(fc2_w.T)  # [d, c]
    cb = np.ascontiguousarray(conv_b.reshape(8, 128).T)  # [128, 8]
    f2b = np.ascontiguousarray(fc2_b.reshape(4, 128).T)  # [128, 4]

    def pmajor(a, lastdim):
        # [B, 4*128, lastdim] -> [B, 128, 4, lastdim] with [s,p,c,:] = a[s, c*128+p]
        return np.ascontiguousarray(a.reshape(B, 4, 128, lastdim).transpose(0, 2, 1, 3))

    xp = pmajor(x, T).astype(np.float16)  # [B, 128, 4, T]
    wet = pmajor(
        np.ascontiguousarray((word_embed + fc1_b[None, None, :]).transpose(0, 2, 1)),
        T,
    )  # [B, 128, 4, T]
    af = img_conv.reshape(B, D, HW)
    afp_full = np.zeros((B, D, HWP), dtype=np.float32)
    afp_full[:, :, :HW] = af
    afp = pmajor(afp_full, HWP)  # [B, 128, 4, HWP]

    in_maps = []
    for c in range(N_CORES):
        sl = slice(c * BL, (c + 1) * BL)
        in_maps.append(
            {
                "x": np.ascontiguousarray(xp[sl]),
                "wet": np.ascontiguousarray(wet[sl]),
                "afp": np.ascontiguousarray(afp[sl]),
                "wt": wt,
                "fc1t": fc1t,
                "fc2t": fc2t,
                "cb": cb,
                "f2b": f2b,
            }
        )
    return in_maps


def _install_ntff_shim():
    """Make run_bass_kernel_spmd(trace=True) work under axon in this image."""
    import types

    if "antenv.axon_hooks" in sys.modules:
        return True
    try:
        m = types.ModuleType("antenv.axon_hooks")
        _hooks = {}

        def set_axon_ntff_profile_hook(h):
            _hooks["h"] = h

        def get_axon_ntff_profile_hook():
            return _hooks.get("h")

        m.set_axon_ntff_profile_hook = set_axon_ntff_profile_hook
        m.get_axon_ntff_profile_hook = get_axon_ntff_profile_hook
        sys.modules["antenv.axon_hooks"] = m
        import antenv

        antenv.axon_hooks = m
        from trn_agent_boot.trn_boot import _ntff_profile_via_ctypes

        hook = _ntff_profile_via_ctypes("/opt/axon/libaxon_pjrt.so")
        set_axon_ntff_profile_hook(hook)
        return hook is not None
    except Exception:
        return False


def kernel(x, word_embed, img_conv, prev_attn=None, conv_v=None, conv_g=None,
           conv_b=None, fc1_w=None, fc1_b=None, fc2_w=None, fc2_b=None):
    if "nc" not in _CACHE:
        _CACHE["nc"] = build_nc()
    nc = _CACHE["nc"]

    in_maps = prep_inputs(
        x, word_embed, img_conv, conv_v, conv_g, conv_b, fc1_w, fc1_b, fc2_w, fc2_b
    )

    trace = bool(os.environ.get("ATTN_BASS_TRACE"))
    if trace:
        trace = _install_ntff_shim()
    res = bass_utils.run_bass_kernel_spmd(
        nc, in_maps, core_ids=list(range(N_CORES)), trace=trace,
        tmpdir=os.environ.get("ATTN_BASS_TMPDIR") or None,
    )
    if trace:
        _CACHE["exec_time_ns"] = res.exec_time_ns
        _CACHE["last_results"] = res

    out = np.concatenate([res.results[i]["out"] for i in range(N_CORES)], axis=0)
    return out.astype(np.float32)


# revision 12
# speedup vs baseline: 1.1912x; 1.1394x over previous
"""Trainium2 Bass kernel for the AttnBlock problem.

Contract: kernel(**inputs) takes the FULL unsharded inputs (numpy, keyed as in
setup_inputs) and returns the FULL output [32, 512, 512] (fp32).

Strategy: data-parallel over batch B=32 across 8 NeuronCores (4 samples/core,
weights replicated). Per sample everything is kept in [feature-on-partition,
t-on-free] layout:
  conv (weight-norm, K=3, f16) -> GLU -> y[c,t] (f16)
  qT[d,t] = fc1T.T @ y + (word_embed + fc1_b)^T    (f16)
  G[n,c]  = af.T @ fc2_w.T   (fc2 folded through the n=196 bottleneck:
            o = fc2 @ ctx^T = G^T @ attnT, saving 8 matmuls/sample)
  scores[t,n] = qT_tile.T @ af[d,n]   (all-f16: n runs at 196, no pad)
  softmax over free dim n, then PE-transpose (f16, 1 cyc/row) -> attnT[n,t]
  o[c,t] = G.T @ attnT ; out = o + fc2_b + y + x
The fc1 matmuls stay f32r x f16-free mix-free: y is f16 so fc1 runs f16.
f32 is kept only where it matters: PSUM accumulation, the residual sum
(yx = y + x in f32), softmax statistics, and the final output.

Performance structure (v3):
  - ~5us of dummy warm-up matmuls at t=0 so the PE HAM clock-gate releases
    (1.2 -> 2.4 GHz) while the head DMAs land.
  - p-major host layouts ([128, 4, T] per sample) -> 4KB contiguous
    per-partition DMA lines; head loads balanced across both HW DMA queues
    (sync + scalar engines), conv-weight pairs split half-and-half.
  - xpad for sample s+1 is prefetched from inside sample s's conv loop so
    the x load never queues behind bulk weight traffic (the v2 trace showed
    a 10us PE stall + HAM re-throttle from exactly that).
  - dense tail: the last sample interleaves the previous sample's O-matmuls,
    G, junk filler matmuls, and the attn transposes between the score tiles
    so the PE never idles long enough to re-trigger the HAM throttle.
  - yx = y + x runs on gpsimd (DVE is the in-sample secondary bottleneck).
"""

import os
import sys

import numpy as np

for _p in ("/opt/trn_rl_repo",):
    if os.path.isdir(_p) and _p not in sys.path:
        sys.path.insert(0, _p)

from contextlib import ExitStack

import concourse.bass as bass
import concourse.tile as tile
from concourse import bacc, mybir
from concourse import bass_utils
from concourse.masks import make_identity

F32 = mybir.dt.float32
F32R = mybir.dt.float32r
F16 = mybir.dt.float16
AF = mybir.ActivationFunctionType
OP = mybir.AluOpType
AX = mybir.AxisListType

B, CIN, T = 32, 512, 512
COUT, KW = 1024, 3
WORD, D = 512, 512
HW = 196
N_CORES = 8
BL = B // N_CORES  # samples per core

_CACHE = {}


def _alloc_xpad(nc, w):
    xpad = w["xpool"].tile([128, 4, T + 2], F16, name="xpad", tag="xpad")
    nc.gpsimd.memset(xpad[:, :, 0:2], 0.0)
    return xpad


def _emit_conv(nc, st, s, w):
    """Input DMAs + conv + GLU -> y, yx for sample s."""
    p = st[s] = {}

    if s == 0:
        xpad = _alloc_xpad(nc, w)
        # head-critical: split x(s0) and the conv-weight pairs across both
        # DMA queues, in pair order (pair i is needed ~5us after pair i-1).
        # All DMA-issue instructions go BEFORE the first sigmoid so the
        # in-order scalar queue never blocks a weight load behind compute.
        nc.sync.dma_start(out=xpad[:, 0:2, 2 : T + 2], in_=w["x_d"][s, :, 0:2, :])
        nc.scalar.dma_start(out=xpad[:, 2:4, 2 : T + 2], in_=w["x_d"][s, :, 2:4, :])
        for i in range(4):
            for ci in range(4):
                eng = nc.sync if ci < 2 else nc.scalar
                eng.dma_start(
                    out=w["wt"][ci][:, i, :, :],
                    in_=w["wt_d"][i, ci * 128 : (ci + 1) * 128, :, :],
                )
    else:
        xpad = w.pop("xpad_next")
    p["xpad"] = xpad

    def load_wet_afp():
        wet = w["wepool"].tile([128, 4, T], F16, name="wet", tag="wet")
        nc.scalar.dma_start(out=wet[:], in_=w["wet_d"][s])
        afp = w["afpool"].tile([128, 4, HW], F16, name="afp", tag="afp")
        nc.scalar.dma_start(out=afp[:], in_=w["afp_d"][s])
        p["wet"] = wet
        p["afp"] = afp

    if s != 0:
        # scalar queue is free in steady state; issue right away
        load_wet_afp()

    if s == 0:
        # after the conv-weight pairs: fc weights + s0 attention inputs on
        # the scalar queue (needed from fc1(0) onward, ~15us later)
        nc.scalar.dma_start(
            out=w["fc1t"][:], in_=w["fc1t_d"].rearrange("(c p) d -> p c d", p=128)
        )
        nc.scalar.dma_start(
            out=w["fc2t"][:], in_=w["fc2t_d"].rearrange("(c p) d -> p c d", p=128)
        )
        load_wet_afp()

    y = w["ypool"].tile([128, 4, T], F16, name="y", tag="y")
    p["y"] = y
    for i in range(4):  # GLU pair: co tile i (a-half) with co tile i+4 (b-half)
        if i == 1 and s < BL - 1:
            # prefetch next sample's x on the sync queue ahead of any bulk
            # traffic emitted later
            nxt = _alloc_xpad(nc, w)
            nc.sync.dma_start(out=nxt[:, :, 2 : T + 2], in_=w["x_d"][s + 1])
            w["xpad_next"] = nxt
        ps_a = w["ps512"].tile([128, T], F32, name="mm", tag="mm")
        ps_b = w["ps512"].tile([128, T], F32, name="mm", tag="mm")
        for half, ps in ((0, ps_a), (1, ps_b)):
            for ci in range(4):
                for k in range(KW):
                    nc.tensor.matmul(
                        ps[:],
                        w["wt"][ci][:, i, k, half * 128 : (half + 1) * 128],
                        xpad[:, ci, k : k + T],
                        start=ci == 0 and k == 0,
                        stop=ci == 3 and k == KW - 1,
                    )
        sig = w["sigpool"].tile([128, T], F32, name="sig", tag="sig")
        nc.scalar.activation(
            sig[:], ps_b[:], AF.Sigmoid, bias=w["cb"][:, i + 4 : i + 5], scale=1.0
        )
        # y_i = (conv_a + bias_a) * sigmoid(conv_b + bias_b)
        nc.vector.scalar_tensor_tensor(
            out=y[:, i, :], in0=ps_a[:], scalar=w["cb"][:, i : i + 1], in1=sig[:],
            op0=OP.add, op1=OP.mult,
        )

    # yx = y + x in f32, on gpsimd (keeps DVE free for the softmax path)
    yx = w["yxpool"].tile([128, 4, T], F32, name="yx", tag="yx")
    for i in range(4):
        nc.gpsimd.tensor_add(yx[:, i, :], y[:, i, :], xpad[:, i, 2 : T + 2])
    p["yx"] = yx


def _emit_fc1(nc, st, s, w):
    """qT[d,t] = fc1T.T @ y + weT (all f16)."""
    p = st[s]
    y, wet = p["y"], p["wet"]
    qt = w["qpool"].tile([128, 4, T], F16, name="qt", tag="qt")
    for dt_ in range(4):
        ps = w["ps512"].tile([128, T], F32, name="mm", tag="mm")
        for cc in range(4):
            nc.tensor.matmul(
                ps[:],
                w["fc1t"][:, cc, dt_ * 128 : (dt_ + 1) * 128],
                y[:, cc, :],
                start=cc == 0,
                stop=cc == 3,
            )
        nc.vector.tensor_add(qt[:, dt_, :], ps[:], wet[:, dt_, :])
    p["qt"] = qt


def _emit_scores_tile(nc, st, s, tt, w):
    """One t-tile of scores + softmax -> normalized attn tile (f16)."""
    p = st[s]
    qt, afp = p["qt"], p["afp"]
    ps_s = w["ps256"].tile([128, HW], F32, name="sc", tag="sc")
    for dd in range(4):
        nc.tensor.matmul(
            ps_s[:],
            qt[:, dd, tt * 128 : (tt + 1) * 128],
            afp[:, dd, :],
            start=dd == 0,
            stop=dd == 3,
        )
    nmax = w["colpool"].tile([128, 1], F32, name="col", tag="col")
    nc.vector.reduce_max(out=nmax[:], in_=ps_s[:], axis=AX.X, negate=True)
    attn_t = w["attnpool"].tile([128, HW], F16, name="attn", tag="attn")
    rsum = w["colpool"].tile([128, 1], F32, name="col", tag="col")
    nc.scalar.activation(
        attn_t[:], ps_s[:], AF.Exp, bias=nmax[:], scale=1.0, accum_out=rsum[:]
    )
    rinv = w["colpool"].tile([128, 1], F32, name="col", tag="col")
    nc.vector.reciprocal(rinv[:], rsum[:])
    nc.vector.tensor_scalar_mul(attn_t[:], attn_t[:], rinv[:])
    p.setdefault("attn", []).append(attn_t)


def _emit_G(nc, st, s, w):
    """G[n,c] = sum_d af[d,n] * fc2T[d,c] (softmax-independent PE filler)."""
    p = st[s]
    afp = p["afp"]
    g_sb = w["gpool"].tile([128, 2, WORD], F16, name="g", tag="g")
    nc.gpsimd.memset(g_sb[64:128, 1, :], 0.0)
    for nch in range(2):
        nsz = 128 if nch == 0 else HW - 128
        g_ps = w["psT"].tile([128, WORD], F32, name="tp", tag="tp")
        for dd in range(4):
            nc.tensor.matmul(
                g_ps[0:nsz, :],
                afp[:, dd, nch * 128 : nch * 128 + nsz],
                w["fc2t"][:, dd, :],
                start=dd == 0,
                stop=dd == 3,
            )
        nc.vector.tensor_copy(g_sb[0:nsz, nch, :], g_ps[0:nsz, :])
    p["g"] = g_sb


def _emit_sample_TC(nc, st, s, w, filler=None):
    """Transpose attn[t,n] -> attnT[n,t] via PE (f16), copy to SBUF."""
    p = st[s]
    attn_tiles = p["attn"]
    tps = [w["psT"].tile([128, T], F16, name="tp", tag="tp") for _ in range(2)]
    for tt in range(4):
        for nch in range(2):
            nsz = 128 if nch == 0 else HW - 128
            nc.tensor.transpose(
                tps[nch][0:nsz, tt * 128 : (tt + 1) * 128],
                attn_tiles[tt][:, nch * 128 : nch * 128 + nsz],
                w["ident"][:],
            )
        if filler is not None and tt < 3:
            filler(1)
    at = w["atpool"].tile([128, 2, T], F16, name="at", tag="at")
    nc.gpsimd.memset(at[64:128, 1, :], 0.0)
    nc.vector.tensor_copy(at[:, 0, :], tps[0][:])
    nc.vector.tensor_copy(at[0 : HW - 128, 1, :], tps[1][0 : HW - 128, :])
    p["at"] = at


def _emit_sample_O(nc, st, s, w, cts=range(4)):
    """o[c,t] = G.T @ attnT ; out = o + fc2_b + (y + x) ; store."""
    p = st[s]
    g_sb, at, yx = p["g"], p["at"], p["yx"]
    for ct in cts:
        ps = w["ps512"].tile([128, T], F32, name="mm", tag="mm")
        for nch in range(2):
            nc.tensor.matmul(
                ps[:],
                g_sb[:, nch, ct * 128 : (ct + 1) * 128],
                at[:, nch, :],
                start=nch == 0,
                stop=nch == 1,
            )
        tmp = w["opool"].tile([128, T], F32, name="tmp", tag="tmp")
        nc.vector.scalar_tensor_tensor(
            out=tmp[:], in0=ps[:], scalar=w["f2b"][:, ct : ct + 1], in1=yx[:, ct, :],
            op0=OP.add, op1=OP.add,
        )
        nc.sync.dma_start(out=w["out_d"][s, ct * 128 : (ct + 1) * 128, :], in_=tmp[:])


def build_nc():
    """Build and compile the per-core Bass program (shared by all 8 cores)."""
    nc = bacc.Bacc("TRN2", target_bir_lowering=False, debug=False, num_devices=N_CORES)
    w = {}
    w["x_d"] = nc.dram_tensor("x", [BL, 128, 4, T], F16, kind="ExternalInput").ap()
    w["wet_d"] = nc.dram_tensor("wet", [BL, 128, 4, T], F16, kind="ExternalInput").ap()
    w["afp_d"] = nc.dram_tensor(
        "afp", [BL, 128, 4, HW], F16, kind="ExternalInput"
    ).ap()
    w["wt_d"] = nc.dram_tensor("wt", [4, CIN, KW, 256], F16, kind="ExternalInput").ap()
    w["fc1t_d"] = nc.dram_tensor("fc1t", [WORD, D], F16, kind="ExternalInput").ap()
    w["fc2t_d"] = nc.dram_tensor("fc2t", [D, WORD], F16, kind="ExternalInput").ap()
    w["cb_d"] = nc.dram_tensor("cb", [128, 8], F32, kind="ExternalInput").ap()
    w["f2b_d"] = nc.dram_tensor("f2b", [128, 4], F32, kind="ExternalInput").ap()
    w["out_d"] = nc.dram_tensor("out", [BL, WORD, T], F32, kind="ExternalOutput").ap()

    with tile.TileContext(nc) as tc, ExitStack() as ctx:
        pool = lambda name, bufs, **kw: ctx.enter_context(
            tc.tile_pool(name=name, bufs=bufs, **kw)
        )
        wpool = pool("wts", 1)
        cpool = pool("consts", 1)
        w["xpool"] = pool("xp", 2)
        w["yxpool"] = pool("yxp", 2)
        w["wepool"] = pool("wep", 2)
        w["afpool"] = pool("afp", 2)
        w["ypool"] = pool("yp", 2)
        w["qpool"] = pool("qp", 1)
        w["gpool"] = pool("gp", 2)
        w["attnpool"] = pool("attnp", 6)
        w["sigpool"] = pool("sigp", 2)
        w["atpool"] = pool("atp", 2)
        w["opool"] = pool("op", 3)
        w["colpool"] = pool("colp", 8)
        w["ps512"] = pool("ps512", 4, space="PSUM")
        w["ps256"] = pool("ps256", 2, space="PSUM")
        w["psT"] = pool("psT", 2, space="PSUM")

        w["wt"] = [
            wpool.tile([128, 4, KW, 256], F16, name=f"wt{c}", tag=f"wt{c}")
            for c in range(4)
        ]
        w["fc1t"] = wpool.tile([128, 4, D], F16, name="fc1t", tag="fc1t")
        w["fc2t"] = wpool.tile([128, 4, WORD], F16, name="fc2t", tag="fc2t")
        w["cb"] = cpool.tile([128, 8], F32, name="cb", tag="cb")
        w["f2b"] = cpool.tile([128, 4], F32, name="f2b", tag="f2b")
        w["ident"] = cpool.tile([128, 128], F16, name="ident", tag="ident")

        # ---- PE warm-up: ~5us of junk matmuls with no DMA dependency so the
        # HAM clock-gate releases (1.2 -> 2.4 GHz) while the head DMAs land.
        mz = cpool.tile([128, T], F32R, name="mz", tag="mz")
        nc.gpsimd.memset(mz[:].bitcast(F32), 0.0)

        def junk_mms(n):
            ps_j = w["ps512"].tile([128, T], F32, name="mm", tag="mm")
            for _ in range(n):
                nc.tensor.matmul(ps_j[:], mz[:, 0:128], mz[:], start=True, stop=True)

        w["junk"] = junk_mms
        junk_mms(12)

        nc.scalar.dma_start(out=w["cb"][:], in_=w["cb_d"][:])
        nc.scalar.dma_start(out=w["f2b"][:], in_=w["f2b_d"][:])
        make_identity(nc, w["ident"][:])

        st = {}
        for s in range(BL):
            _emit_conv(nc, st, s, w)
            _emit_fc1(nc, st, s, w)
            if s < BL - 1:
                for tt in range(4):
                    _emit_scores_tile(nc, st, s, tt, w)
                _emit_G(nc, st, s, w)
                if s > 0:
                    _emit_sample_O(nc, st, s - 1, w)
                _emit_sample_TC(nc, st, s, w)
            else:
                # dense tail: interleave the previous sample's O-matmuls, G,
                # and junk filler between the score tiles / transposes so the
                # PE stays busy under the softmax latency (prevents the HAM
                # re-throttle and per-transpose stalls at program end).
                _emit_scores_tile(nc, st, s, 0, w)
                _emit_scores_tile(nc, st, s, 1, w)
                _emit_sample_O(nc, st, s - 1, w, cts=(0, 1))
                _emit_scores_tile(nc, st, s, 2, w)
                _emit_sample_O(nc, st, s - 1, w, cts=(2,))
                _emit_scores_tile(nc, st, s, 3, w)
                _emit_sample_O(nc, st, s - 1, w, cts=(3,))
                _emit_G(nc, st, s, w)
                junk_mms(2)
                _emit_sample_TC(nc, st, s, w, filler=junk_mms)
        _emit_sample_O(nc, st, BL - 1, w)

    nc.compile()
    return nc


def prep_inputs(x, word_embed, img_conv, conv_v, conv_g, conv_b, fc1_w, fc1_b, fc2_w, fc2_b):
    """Host-side weight-norm + p-major layout prep. Returns per-core input maps."""
    x = np.asarray(x, dtype=np.float32)
    word_embed = np.asarray(word_embed, dtype=np.float32)
    img_conv = np.asarray(img_conv, dtype=np.float32)
    conv_v = np.asarray(conv_v, dtype=np.float32)
    conv_g = np.asarray(conv_g, dtype=np.float32)
    conv_b = np.asarray(conv_b, dtype=np.float32)
    fc1_w = np.asarray(fc1_w, dtype=np.float32)
    fc1_b = np.asarray(fc1_b, dtype=np.float32)
    fc2_w = np.asarray(fc2_w, dtype=np.float32)
    fc2_b = np.asarray(fc2_b, dtype=np.float32)

    v_norm = np.sqrt(np.sum(conv_v * conv_v, axis=(1, 2), keepdims=True))
    wconv = conv_g[:, None, None] * conv_v / v_norm  # [COUT, CIN, KW]
    wtf = wconv.transpose(1, 2, 0).astype(np.float16)  # [CIN, KW, COUT]
    wt = np.ascontiguousarray(
        np.stack(
            [
                np.concatenate(
                    [wtf[:, :, i * 128 : (i + 1) * 128],
                     wtf[:, :, (i + 4) * 128 : (i + 5) * 128]],
                    axis=-1,
                )
                for i in range(4)
            ]
        )
    )  # [4, CIN, KW, 256] pair-major
    fc1t = np.ascontiguousarray(fc1_w.T).astype(np.float16)  # [c, d]
    fc2t = np.ascontiguousarray(fc2_w.T).astype(np.float16)  # [d, c]
    cb = np.ascontiguousarray(conv_b.reshape(8, 128).T)  # [128, 8]
    f2b = np.ascontiguousarray(fc2_b.reshape(4, 128).T)  # [128, 4]

    def pmajor(a, lastdim):
        # [B, 4*128, lastdim] -> [B, 128, 4, lastdim] with [s,p,c,:] = a[s, c*128+p]
        return np.ascontiguousarray(a.reshape(B, 4, 128, lastdim).transpose(0, 2, 1, 3))

    xp = pmajor(x, T).astype(np.float16)  # [B, 128, 4, T]
    wet = pmajor(
        np.ascontiguousarray((word_embed + fc1_b[None, None, :]).transpose(0, 2, 1)),
        T,
    ).astype(np.float16)  # [B, 128, 4, T]
    af = img_conv.reshape(B, D, HW)
    afp = pmajor(af, HW).astype(np.float16)  # [B, 128, 4, HW]

    in_maps = []
    for c in range(N_CORES):
        sl = slice(c * BL, (c + 1) * BL)
        in_maps.append(
            {
                "x": np.ascontiguousarray(xp[sl]),
                "wet": np.ascontiguousarray(wet[sl]),
                "afp": np.ascontiguousarray(afp[sl]),
                "wt": wt,
                "fc1t": fc1t,
                "fc2t": fc2t,
                "cb": cb,
                "f2b": f2b,
            }
        )
    return in_maps


def _install_ntff_shim():
    """Make run_bass_kernel_spmd(trace=True) work under axon in this image."""
    import types

    if "antenv.axon_hooks" in sys.modules:
        return True
    try:
        m = types.ModuleType("antenv.axon_hooks")
        _hooks = {}

        def set_axon_ntff_profile_hook(h):
            _hooks["h"] = h

        def get_axon_ntff_profile_hook():
            return _hooks.get("h")

        m.set_axon_ntff_profile_hook = set_axon_ntff_profile_hook
        m.get_axon_ntff_profile_hook = get_axon_ntff_profile_hook
        sys.modules["antenv.axon_hooks"] = m
        import antenv

        antenv.axon_hooks = m
        from trn_agent_boot.trn_boot import _ntff_profile_via_ctypes

        hook = _ntff_profile_via_ctypes("/opt/axon/libaxon_pjrt.so")
        set_axon_ntff_profile_hook(hook)
        return hook is not None
    except Exception:
        return False


def kernel(x, word_embed, img_conv, prev_attn=None, conv_v=None, conv_g=None,
           conv_b=None, fc1_w=None, fc1_b=None, fc2_w=None, fc2_b=None):
    if "nc" not in _CACHE:
        _CACHE["nc"] = build_nc()
    nc = _CACHE["nc"]

    in_maps = prep_inputs(
        x, word_embed, img_conv, conv_v, conv_g, conv_b, fc1_w, fc1_b, fc2_w, fc2_b
    )

    trace = bool(os.environ.get("ATTN_BASS_TRACE"))
    if trace:
        trace = _install_ntff_shim()
    res = bass_utils.run_bass_kernel_spmd(
        nc, in_maps, core_ids=list(range(N_CORES)), trace=trace,
        tmpdir=os.environ.get("ATTN_BASS_TMPDIR") or None,
    )
    if trace:
        _CACHE["exec_time_ns"] = res.exec_time_ns
        _CACHE["last_results"] = res

    out = np.concatenate([res.results[i]["out"] for i in range(N_CORES)], axis=0)
    return out.astype(np.float32)


# revision 14
# speedup vs baseline: 1.2754x; 1.0707x over previous
"""Trainium2 Bass kernel for the AttnBlock problem.

Contract: kernel(**inputs) takes the FULL unsharded inputs (numpy, keyed as in
setup_inputs) and returns the FULL output [32, 512, 512] (fp32).

Strategy: data-parallel over batch B=32 across 8 NeuronCores (4 samples/core,
weights replicated). Per sample everything is kept in [feature-on-partition,
t-on-free] layout:
  conv (weight-norm, K=3, f16) -> GLU -> y[c,t] (f16)
  qT[d,t] = fc1T.T @ y + (word_embed + fc1_b)^T    (f16)
  G[n,c]  = af.T @ fc2_w.T   (fc2 folded through the n=196 bottleneck:
            o = fc2 @ ctx^T = G^T @ attnT, saving 8 matmuls/sample)
  scores[t,n] = qT_tile.T @ af[d,n]   (all-f16: n runs at 196, no pad)
  softmax over free dim n, then PE-transpose (f16, 1 cyc/row) -> attnT[n,t]
  o[c,t] = G.T @ attnT ; out = o + fc2_b + y + x
The fc1 matmuls stay f32r x f16-free mix-free: y is f16 so fc1 runs f16.
f32 is kept only where it matters: PSUM accumulation, the residual sum
(yx = y + x in f32), softmax statistics, and the final output.

Performance structure (v3):
  - ~5us of dummy warm-up matmuls at t=0 so the PE HAM clock-gate releases
    (1.2 -> 2.4 GHz) while the head DMAs land.
  - p-major host layouts ([128, 4, T] per sample) -> 4KB contiguous
    per-partition DMA lines; head loads balanced across both HW DMA queues
    (sync + scalar engines), conv-weight pairs split half-and-half.
  - xpad for sample s+1 is prefetched from inside sample s's conv loop so
    the x load never queues behind bulk weight traffic (the v2 trace showed
    a 10us PE stall + HAM re-throttle from exactly that).
  - dense tail: the last sample interleaves the previous sample's O-matmuls,
    G, junk filler matmuls, and the attn transposes between the score tiles
    so the PE never idles long enough to re-trigger the HAM throttle.
  - yx = y + x runs on gpsimd (DVE is the in-sample secondary bottleneck).
"""

import os
import sys

import numpy as np

for _p in ("/opt/trn_rl_repo",):
    if os.path.isdir(_p) and _p not in sys.path:
        sys.path.insert(0, _p)

from contextlib import ExitStack

import concourse.bass as bass
import concourse.tile as tile
from concourse import bacc, mybir
from concourse import bass_utils
from concourse.masks import make_identity

F32 = mybir.dt.float32
F32R = mybir.dt.float32r
F16 = mybir.dt.float16
AF = mybir.ActivationFunctionType
OP = mybir.AluOpType
AX = mybir.AxisListType

B, CIN, T = 32, 512, 512
COUT, KW = 1024, 3
WORD, D = 512, 512
HW = 196
N_CORES = 8
BL = B // N_CORES  # samples per core

_CACHE = {}


def _alloc_xpad(nc, w):
    xpad = w["xpool"].tile([128, 4, T + 2], F16, name="xpad", tag="xpad")
    nc.gpsimd.memset(xpad[:, :, 0:2], 0.0)
    return xpad


def _emit_conv(nc, st, s, w):
    """Input DMAs + conv + GLU -> y, yx for sample s."""
    p = st[s] = {}

    if s == 0:
        xpad = _alloc_xpad(nc, w)
        # head-critical: split x(s0) and the conv-weight pairs across both
        # DMA queues, in pair order (pair i is needed ~5us after pair i-1).
        # All DMA-issue instructions go BEFORE the first sigmoid so the
        # in-order scalar queue never blocks a weight load behind compute.
        nc.sync.dma_start(out=xpad[:, 0:2, 2 : T + 2], in_=w["x_d"][s, :, 0:2, :])
        nc.scalar.dma_start(out=xpad[:, 2:4, 2 : T + 2], in_=w["x_d"][s, :, 2:4, :])
        for i in range(4):
            for ci in range(4):
                eng = nc.sync if ci < 2 else nc.scalar
                eng.dma_start(
                    out=w["wt"][ci][:, i, :, :],
                    in_=w["wt_d"][i, ci * 128 : (ci + 1) * 128, :, :],
                )
    else:
        xpad = w.pop("xpad_next")
    p["xpad"] = xpad

    def load_wet_afp():
        afp = w["afpool"].tile([128, 4, HW], F16, name="afp", tag="afp")
        nc.scalar.dma_start(out=afp[:], in_=w["afp_d"][s])
        wet = w["wepool"].tile([128, 4, T], F16, name="wet", tag="wet")
        nc.scalar.dma_start(out=wet[:], in_=w["wet_d"][s])
        p["wet"] = wet
        p["afp"] = afp

    if s != 0:
        # scalar queue is free in steady state; issue right away
        load_wet_afp()

    if s == 0:
        # after the conv-weight pairs: fc weights + s0 attention inputs on
        # the scalar queue (needed from fc1(0) onward, ~15us later)
        nc.scalar.dma_start(out=w["fc1n"][:], in_=w["fc1n_d"])
        nc.scalar.dma_start(
            out=w["fc2t"][:], in_=w["fc2t_d"].rearrange("(c p) d -> p c d", p=128)
        )
        load_wet_afp()

    y = w["ypool"].tile([128, 4, T], F16, name="y", tag="y")
    p["y"] = y
    for i in range(4):  # GLU pair: co tile i (a-half) with co tile i+4 (b-half)
        if i == 1 and s < BL - 1:
            # prefetch next sample's x on the sync queue ahead of any bulk
            # traffic emitted later
            nxt = _alloc_xpad(nc, w)
            nc.sync.dma_start(out=nxt[:, :, 2 : T + 2], in_=w["x_d"][s + 1])
            w["xpad_next"] = nxt
        ps_a = w["ps512"].tile([128, T], F32, name="mm", tag="mm")
        ps_b = w["ps512"].tile([128, T], F32, name="mm", tag="mm")
        for half, ps in ((0, ps_a), (1, ps_b)):
            for ci in range(4):
                for k in range(KW):
                    nc.tensor.matmul(
                        ps[:],
                        w["wt"][ci][:, i, k, half * 128 : (half + 1) * 128],
                        xpad[:, ci, k : k + T],
                        start=ci == 0 and k == 0,
                        stop=ci == 3 and k == KW - 1,
                    )
        sig = w["sigpool"].tile([128, T], F16, name="sig", tag="sig")
        nc.scalar.activation(
            sig[:], ps_b[:], AF.Sigmoid, bias=w["cb"][:, i + 4 : i + 5], scale=1.0
        )
        # y_i = (conv_a + bias_a) * sigmoid(conv_b + bias_b)
        nc.vector.scalar_tensor_tensor(
            out=y[:, i, :], in0=ps_a[:], scalar=w["cb"][:, i : i + 1], in1=sig[:],
            op0=OP.add, op1=OP.mult,
        )

    # yx = y + x in f32, on gpsimd (keeps DVE free for the softmax path)
    yx = w["yxpool"].tile([128, 4, T], F32, name="yx", tag="yx")
    for i in range(4):
        nc.gpsimd.tensor_add(yx[:, i, :], y[:, i, :], xpad[:, i, 2 : T + 2])
    p["yx"] = yx


def _emit_M(nc, st, s, w):
    """M[c,n] = sum_d fc1_w[d,c] * af[d,n]: the fc1 weights folded through
    the n=196 bottleneck. scores = y^T M + we^T af then needs no separate
    fc1 pass and no DVE q-adds."""
    p = st[s]
    afp = p["afp"]
    m_sb = w["mpool"].tile([128, 4, HW], F16, name="m", tag="m")
    for ct in range(4):
        ps = w["ps256"].tile([128, HW], F32, name="sc", tag="sc")
        for dd in range(4):
            nc.tensor.matmul(
                ps[:],
                w["fc1n"][:, dd, ct * 128 : (ct + 1) * 128],
                afp[:, dd, :],
                start=dd == 0,
                stop=dd == 3,
            )
        nc.vector.tensor_copy(m_sb[:, ct, :], ps[:])
    p["m"] = m_sb


def _emit_scores_tile(nc, st, s, tt, w):
    """One t-tile of scores + softmax -> normalized attn tile (f16)."""
    p = st[s]
    y, wet, afp, m_sb = p["y"], p["wet"], p["afp"], p["m"]
    ps_s = w["ps256"].tile([128, HW], F32, name="sc", tag="sc")
    for dd in range(4):
        nc.tensor.matmul(
            ps_s[:],
            wet[:, dd, tt * 128 : (tt + 1) * 128],
            afp[:, dd, :],
            start=dd == 0,
            stop=False,
        )
    for cc in range(4):
        nc.tensor.matmul(
            ps_s[:],
            y[:, cc, tt * 128 : (tt + 1) * 128],
            m_sb[:, cc, :],
            start=False,
            stop=cc == 3,
        )
    nmax = w["colpool"].tile([128, 1], F32, name="col", tag="col")
    nc.vector.reduce_max(out=nmax[:], in_=ps_s[:], axis=AX.X, negate=True)
    attn_t = w["attnpool"].tile([128, HW], F16, name="attn", tag="attn")
    rsum = w["colpool"].tile([128, 1], F32, name="col", tag="col")
    nc.scalar.activation(
        attn_t[:], ps_s[:], AF.Exp, bias=nmax[:], scale=1.0, accum_out=rsum[:]
    )
    rinv = w["colpool"].tile([128, 1], F32, name="col", tag="col")
    nc.vector.reciprocal(rinv[:], rsum[:])
    nc.vector.tensor_scalar_mul(attn_t[:], attn_t[:], rinv[:])
    p.setdefault("attn", []).append(attn_t)


def _emit_G(nc, st, s, w):
    """G[n,c] = sum_d af[d,n] * fc2T[d,c] (softmax-independent PE filler)."""
    p = st[s]
    afp = p["afp"]
    g_sb = w["gpool"].tile([128, 2, WORD], F16, name="g", tag="g")
    nc.gpsimd.memset(g_sb[64:128, 1, :], 0.0)
    for nch in range(2):
        nsz = 128 if nch == 0 else HW - 128
        g_ps = w["psT"].tile([128, WORD], F32, name="tp", tag="tp")
        for dd in range(4):
            nc.tensor.matmul(
                g_ps[0:nsz, :],
                afp[:, dd, nch * 128 : nch * 128 + nsz],
                w["fc2t"][:, dd, :],
                start=dd == 0,
                stop=dd == 3,
            )
        nc.vector.tensor_copy(g_sb[0:nsz, nch, :], g_ps[0:nsz, :])
    p["g"] = g_sb


def _emit_sample_TC(nc, st, s, w, filler=None):
    """Transpose attn[t,n] -> attnT[n,t] via PE (f16), copy to SBUF."""
    p = st[s]
    attn_tiles = p["attn"]
    tps = [w["psT"].tile([128, T], F16, name="tp", tag="tp") for _ in range(2)]
    for tt in range(4):
        for nch in range(2):
            nsz = 128 if nch == 0 else HW - 128
            nc.tensor.transpose(
                tps[nch][0:nsz, tt * 128 : (tt + 1) * 128],
                attn_tiles[tt][:, nch * 128 : nch * 128 + nsz],
                w["ident"][:],
            )
        if filler is not None and tt < 3:
            filler(1)
    at = w["atpool"].tile([128, 2, T], F16, name="at", tag="at")
    nc.gpsimd.memset(at[64:128, 1, :], 0.0)
    nc.vector.tensor_copy(at[:, 0, :], tps[0][:])
    nc.vector.tensor_copy(at[0 : HW - 128, 1, :], tps[1][0 : HW - 128, :])
    p["at"] = at


def _emit_sample_O(nc, st, s, w, cts=range(4)):
    """o[c,t] = G.T @ attnT ; out = o + fc2_b + (y + x) ; store."""
    p = st[s]
    g_sb, at, yx = p["g"], p["at"], p["yx"]
    for ct in cts:
        ps = w["ps512"].tile([128, T], F32, name="mm", tag="mm")
        for nch in range(2):
            nc.tensor.matmul(
                ps[:],
                g_sb[:, nch, ct * 128 : (ct + 1) * 128],
                at[:, nch, :],
                start=nch == 0,
                stop=nch == 1,
            )
        tmp = w["opool"].tile([128, T], F32, name="tmp", tag="tmp")
        nc.vector.scalar_tensor_tensor(
            out=tmp[:], in0=ps[:], scalar=w["f2b"][:, ct : ct + 1], in1=yx[:, ct, :],
            op0=OP.add, op1=OP.add,
        )
        nc.sync.dma_start(out=w["out_d"][s, ct * 128 : (ct + 1) * 128, :], in_=tmp[:])


def build_nc():
    """Build and compile the per-core Bass program (shared by all 8 cores)."""
    nc = bacc.Bacc("TRN2", target_bir_lowering=False, debug=False, num_devices=N_CORES)
    w = {}
    w["x_d"] = nc.dram_tensor("x", [BL, 128, 4, T], F16, kind="ExternalInput").ap()
    w["wet_d"] = nc.dram_tensor("wet", [BL, 128, 4, T], F16, kind="ExternalInput").ap()
    w["afp_d"] = nc.dram_tensor(
        "afp", [BL, 128, 4, HW], F16, kind="ExternalInput"
    ).ap()
    w["wt_d"] = nc.dram_tensor("wt", [4, CIN, KW, 256], F16, kind="ExternalInput").ap()
    w["fc1n_d"] = nc.dram_tensor("fc1n", [128, 4, WORD], F16, kind="ExternalInput").ap()
    w["fc2t_d"] = nc.dram_tensor("fc2t", [D, WORD], F16, kind="ExternalInput").ap()
    w["cb_d"] = nc.dram_tensor("cb", [128, 8], F32, kind="ExternalInput").ap()
    w["f2b_d"] = nc.dram_tensor("f2b", [128, 4], F32, kind="ExternalInput").ap()
    w["out_d"] = nc.dram_tensor("out", [BL, WORD, T], F32, kind="ExternalOutput").ap()

    with tile.TileContext(nc) as tc, ExitStack() as ctx:
        pool = lambda name, bufs, **kw: ctx.enter_context(
            tc.tile_pool(name=name, bufs=bufs, **kw)
        )
        wpool = pool("wts", 1)
        cpool = pool("consts", 1)
        w["xpool"] = pool("xp", 3)
        w["yxpool"] = pool("yxp", 2)
        w["wepool"] = pool("wep", 3)
        w["afpool"] = pool("afp", 3)
        w["ypool"] = pool("yp", 2)
        w["mpool"] = pool("mp", 2)
        w["gpool"] = pool("gp", 2)
        w["attnpool"] = pool("attnp", 8)
        w["sigpool"] = pool("sigp", 2)
        w["atpool"] = pool("atp", 2)
        w["opool"] = pool("op", 5)
        w["colpool"] = pool("colp", 12)
        w["ps512"] = pool("ps512", 4, space="PSUM")
        w["ps256"] = pool("ps256", 2, space="PSUM")
        w["psT"] = pool("psT", 2, space="PSUM")

        w["wt"] = [
            wpool.tile([128, 4, KW, 256], F16, name=f"wt{c}", tag=f"wt{c}")
            for c in range(4)
        ]
        w["fc1n"] = wpool.tile([128, 4, WORD], F16, name="fc1n", tag="fc1n")
        w["fc2t"] = wpool.tile([128, 4, WORD], F16, name="fc2t", tag="fc2t")
        w["cb"] = cpool.tile([128, 8], F32, name="cb", tag="cb")
        w["f2b"] = cpool.tile([128, 4], F32, name="f2b", tag="f2b")
        w["ident"] = cpool.tile([128, 128], F16, name="ident", tag="ident")

        # ---- PE warm-up: ~5us of junk matmuls with no DMA dependency so the
        # HAM clock-gate releases (1.2 -> 2.4 GHz) while the head DMAs land.
        mz = cpool.tile([128, T], F32R, name="mz", tag="mz")
        nc.gpsimd.memset(mz[:].bitcast(F32), 0.0)

        def junk_mms(n):
            ps_j = w["ps512"].tile([128, T], F32, name="mm", tag="mm")
            for _ in range(n):
                nc.tensor.matmul(ps_j[:], mz[:, 0:128], mz[:], start=True, stop=True)

        w["junk"] = junk_mms
        junk_mms(12)

        nc.scalar.dma_start(out=w["cb"][:], in_=w["cb_d"][:])
        nc.scalar.dma_start(out=w["f2b"][:], in_=w["f2b_d"][:])
        make_identity(nc, w["ident"][:])

        st = {}
        for s in range(BL):
            _emit_conv(nc, st, s, w)
            _emit_M(nc, st, s, w)
            if s < BL - 1:
                for tt in range(4):
                    _emit_scores_tile(nc, st, s, tt, w)
                _emit_G(nc, st, s, w)
                if s > 0:
                    _emit_sample_O(nc, st, s - 1, w)
                _emit_sample_TC(nc, st, s, w)
            else:
                # dense tail: interleave the previous sample's O-matmuls, G,
                # and junk filler between the score tiles / transposes so the
                # PE stays busy under the softmax latency (prevents the HAM
                # re-throttle and per-transpose stalls at program end).
                _emit_scores_tile(nc, st, s, 0, w)
                _emit_scores_tile(nc, st, s, 1, w)
                _emit_sample_O(nc, st, s - 1, w, cts=(0, 1))
                _emit_scores_tile(nc, st, s, 2, w)
                _emit_sample_O(nc, st, s - 1, w, cts=(2,))
                _emit_scores_tile(nc, st, s, 3, w)
                _emit_sample_O(nc, st, s - 1, w, cts=(3,))
                _emit_G(nc, st, s, w)
                junk_mms(2)
                _emit_sample_TC(nc, st, s, w, filler=junk_mms)
        _emit_sample_O(nc, st, BL - 1, w)

    nc.compile()
    return nc


def prep_inputs(x, word_embed, img_conv, conv_v, conv_g, conv_b, fc1_w, fc1_b, fc2_w, fc2_b):
    """Host-side weight-norm + p-major layout prep. Returns per-core input maps."""
    x = np.asarray(x, dtype=np.float32)
    word_embed = np.asarray(word_embed, dtype=np.float32)
    img_conv = np.asarray(img_conv, dtype=np.float32)
    conv_v = np.asarray(conv_v, dtype=np.float32)
    conv_g = np.asarray(conv_g, dtype=np.float32)
    conv_b = np.asarray(conv_b, dtype=np.float32)
    fc1_w = np.asarray(fc1_w, dtype=np.float32)
    fc1_b = np.asarray(fc1_b, dtype=np.float32)
    fc2_w = np.asarray(fc2_w, dtype=np.float32)
    fc2_b = np.asarray(fc2_b, dtype=np.float32)

    v_norm = np.sqrt(np.sum(conv_v * conv_v, axis=(1, 2), keepdims=True))
    wconv = conv_g[:, None, None] * conv_v / v_norm  # [COUT, CIN, KW]
    wtf = wconv.transpose(1, 2, 0).astype(np.float16)  # [CIN, KW, COUT]
    wt = np.ascontiguousarray(
        np.stack(
            [
                np.concatenate(
                    [wtf[:, :, i * 128 : (i + 1) * 128],
                     wtf[:, :, (i + 4) * 128 : (i + 5) * 128]],
                    axis=-1,
                )
                for i in range(4)
            ]
        )
    )  # [4, CIN, KW, 256] pair-major
    fc1n = np.ascontiguousarray(
        fc1_w.reshape(4, 128, WORD).transpose(1, 0, 2)
    ).astype(np.float16)  # [128, 4, c]: [p, dd, c] = fc1_w[dd*128+p, c]
    fc2t = np.ascontiguousarray(fc2_w.T).astype(np.float16)  # [d, c]
    cb = np.ascontiguousarray(conv_b.reshape(8, 128).T)  # [128, 8]
    f2b = np.ascontiguousarray(fc2_b.reshape(4, 128).T)  # [128, 4]

    def pmajor(a, lastdim):
        # [B, 4*128, lastdim] -> [B, 128, 4, lastdim] with [s,p,c,:] = a[s, c*128+p]
        return np.ascontiguousarray(a.reshape(B, 4, 128, lastdim).transpose(0, 2, 1, 3))

    xp = pmajor(x, T).astype(np.float16)  # [B, 128, 4, T]
    wet = pmajor(
        np.ascontiguousarray((word_embed + fc1_b[None, None, :]).transpose(0, 2, 1)),
        T,
    ).astype(np.float16)  # [B, 128, 4, T]
    af = img_conv.reshape(B, D, HW)
    afp = pmajor(af, HW).astype(np.float16)  # [B, 128, 4, HW]

    in_maps = []
    for c in range(N_CORES):
        sl = slice(c * BL, (c + 1) * BL)
        in_maps.append(
            {
                "x": np.ascontiguousarray(xp[sl]),
                "wet": np.ascontiguousarray(wet[sl]),
                "afp": np.ascontiguousarray(afp[sl]),
                "wt": wt,
                "fc1n": fc1n,
                "fc2t": fc2t,
                "cb": cb,
                "f2b": f2b,
            }
        )
    return in_maps


def _install_ntff_shim():
    """Make run_bass_kernel_spmd(trace=True) work under axon in this image."""
    import types

    if "antenv.axon_hooks" in sys.modules:
        return True
    try:
        m = types.ModuleType("antenv.axon_hooks")
        _hooks = {}

        def set_axon_ntff_profile_hook(h):
            _hooks["h"] = h

        def get_axon_ntff_profile_hook():
            return _hooks.get("h")

        m.set_axon_ntff_profile_hook = set_axon_ntff_profile_hook
        m.get_axon_ntff_profile_hook = get_axon_ntff_profile_hook
        sys.modules["antenv.axon_hooks"] = m
        import antenv

        antenv.axon_hooks = m
        from trn_agent_boot.trn_boot import _ntff_profile_via_ctypes

        hook = _ntff_profile_via_ctypes("/opt/axon/libaxon_pjrt.so")
        set_axon_ntff_profile_hook(hook)
        return hook is not None
    except Exception:
        return False


def kernel(x, word_embed, img_conv, prev_attn=None, conv_v=None, conv_g=None,
           conv_b=None, fc1_w=None, fc1_b=None, fc2_w=None, fc2_b=None):
    if "nc" not in _CACHE:
        _CACHE["nc"] = build_nc()
    nc = _CACHE["nc"]

    in_maps = prep_inputs(
        x, word_embed, img_conv, conv_v, conv_g, conv_b, fc1_w, fc1_b, fc2_w, fc2_b
    )

    trace = bool(os.environ.get("ATTN_BASS_TRACE"))
    if trace:
        trace = _install_ntff_shim()
    res = bass_utils.run_bass_kernel_spmd(
        nc, in_maps, core_ids=list(range(N_CORES)), trace=trace,
        tmpdir=os.environ.get("ATTN_BASS_TMPDIR") or None,
    )
    if trace:
        _CACHE["exec_time_ns"] = res.exec_time_ns
        _CACHE["last_results"] = res

    out = np.concatenate([res.results[i]["out"] for i in range(N_CORES)], axis=0)
    return out.astype(np.float32)
